# revision 7
# baseline (speedup 1.0000x reference)
"""Trainium2 Bass kernel for the argmax-distance-weighted loss.

loss = sum_b sum_{j,k} ((jstar_b - j)^2 + (kstar_b - k)^2) * t[b,j,k]
where (jstar_b, kstar_b) is the (first-occurrence) argmax location of t[b].

Decomposition used per batch b:
    loss_b = (jstar^2 + kstar^2)*S - 2*jstar*Sj - 2*kstar*Sk + Sj2 + Sk2
with S    = sum t[b]
     Sj   = sum_j j   * rowsum[b, j]      rowsum[b,j] = sum_k t[b,j,k]
     Sj2  = sum_j j^2 * rowsum[b, j]
     Sk   = sum_k k   * colsum[b, k]      colsum[b,k] = sum_j t[b,j,k]
     Sk2  = sum_k k^2 * colsum[b, k]

Device (8 NeuronCores, data-parallel over batch): per 128-batch tile the
DVE does three full reduction passes (rowsum, colsum contiguous/strided,
rowmax) plus tiny fused weighted reductions, emitting 8 moments per batch.
jstar is recovered exactly (first row whose rowmax equals the batch max).
Host: gathers row jstar per batch (64 floats) to resolve kstar with exact
first-occurrence semantics, then evaluates the closed form and sums.
"""

import os
import sys

import numpy as np

try:
    import concourse.bass as bass
except ModuleNotFoundError:  # make concourse importable in a bare container
    for _p in ("/opt/trn_rl_repo", "/root/.axon_site/_ro/trn_rl_repo"):
        if os.path.isdir(_p) and _p not in sys.path:
            sys.path.insert(0, _p)
    import concourse.bass as bass

import concourse.mybir as mybir
from concourse.bass_utils import run_bass_kernel_spmd
from concourse.tile import TileContext
# --- workaround: this walrus build encodes only ONE sync-wait per TPB ---
# instruction. Tile attaches several waits to one instruction (tail drain,
# DMA copies, ...), which codegen rejects with "Too many sync wait
# commands". Post-pass: hoist all but the last wait of each instruction
# into standalone same-engine NoOps placed immediately before it.


def _split_multiwait_instructions(nc: bass.Bass) -> None:
    # (bb, inst-name) pairs needing surgery
    targets = []
    for fn in nc.m.functions:
        for bb in fn.blocks:
            for inst in bb.instructions:
                si = inst.sync_info
                if si is not None and len(si.on_wait) > 1:
                    targets.append((bb, inst.name))
    if not targets:
        return

    moved_nop_names: set[str] = set()
    plan: dict[str, list] = {}  # target-inst-name -> nop instructions
    for bb, iname in targets:
        inst = next(i for i in bb.instructions if i.name == iname)
        waits = list(inst.sync_info.on_wait)
        inst.sync_info.on_wait = waits[-1:]
        nops = []
        for w in waits[:-1]:
            bi = nc.engines[inst.engine].nop(nofuse=True, hint="split_wait")
            bi.ins.sync_info = mybir.SyncInfo(on_wait=[w], on_update=[])
            nops.append(bi.ins)
            moved_nop_names.add(bi.ins.name)
        plan[iname] = nops

    # relocate the nops to sit immediately before their target instruction
    for fn in nc.m.functions:
        for bb in fn.blocks:
            insts = list(bb.instructions)
            kept = [i for i in insts if i.name not in moved_nop_names]
            out: list = []
            changed = len(kept) != len(insts)
            for inst in kept:
                if inst.name in plan:
                    out.extend(plan[inst.name])
                    changed = True
                out.append(inst)
            if changed:
                bb.instructions = out

B, H, W = 8192, 64, 64
NCORES = 8
P = 128  # SBUF partitions

F32 = mybir.dt.float32
Alu = mybir.AluOpType
Ax = mybir.AxisListType

# output layout per (partition, tile): 8 fp32 moments
Q_M, Q_S, Q_SJ, Q_SJ2, Q_SK, Q_SK2, Q_RJ, Q_SPARE = range(8)


def build(bpc: int) -> bass.Bass:
    """Build the per-core Bass program for `bpc` batches per core."""
    ntiles = bpc // P
    assert ntiles * P == bpc

    nc = bass.Bass()
    x = nc.declare_dram_parameter("x", [bpc, H, W], F32, isOutput=False)
    wc = nc.declare_dram_parameter("wconsts", [3, W], F32, isOutput=False)
    out = nc.declare_dram_parameter("moments", [P, ntiles * 8], F32, isOutput=True)

    with TileContext(nc) as tc:
        with (
            tc.tile_pool(name="xpool", bufs=3) as xpool,
            tc.tile_pool(name="consts", bufs=1) as cpool,
            tc.tile_pool(name="inter", bufs=1) as ipool,
            tc.tile_pool(name="small", bufs=4) as spool,
        ):
            # Broadcast the weight constants [3, W] across all partitions.
            wtile = cpool.tile([P, 3, W], F32)
            wc_ap = wc[:, :]
            bcast = bass.AP(
                tensor=wc_ap.tensor,
                offset=wc_ap.offset,
                ap=[[0, P]] + list(wc_ap.ap),
            )
            nc.sync.dma_start(out=wtile, in_=bcast)
            w1 = wtile[:, 0, :]  # 0..63
            wr = wtile[:, 2, :]  # 64-j

            outs = ipool.tile([P, ntiles * 8], F32)
            nc.vector.memset(outs, 0.0)

            for t in range(ntiles):
                xt = xpool.tile([P, H, W], F32)
                nc.sync.dma_start(out=xt, in_=x[t * P : (t + 1) * P, :, :])

                rs = spool.tile([P, H], F32, tag="rs")
                cs = spool.tile([P, W], F32, tag="cs")
                rm = spool.tile([P, H], F32, tag="rm")
                scr = spool.tile([P, H], F32, tag="scr")
                scr2 = spool.tile([P, H], F32, tag="scr2")

                # full-data passes
                nc.vector.tensor_reduce(out=rs, in_=xt[:, :, :], axis=Ax.X, op=Alu.add)
                nc.vector.tensor_reduce(out=rm, in_=xt[:, :, :], axis=Ax.X, op=Alu.max)
                xk = xt[:, :, :].rearrange("p j k -> p k j")
                nc.vector.tensor_reduce(out=cs, in_=xk, axis=Ax.X, op=Alu.add)

                def o(q, t=t):
                    return outs[:, t * 8 + q : t * 8 + q + 1]

                # batch max / total sum
                nc.vector.tensor_reduce(out=o(Q_M), in_=rm, axis=Ax.X, op=Alu.max)
                nc.vector.tensor_reduce(out=o(Q_S), in_=rs, axis=Ax.X, op=Alu.add)
                # weighted moments: out = (in0 * 1.0) * w; accum_out = sum(out)
                nc.vector.scalar_tensor_tensor(
                    out=scr, in0=rs, scalar=1.0, in1=w1,
                    op0=Alu.mult, op1=Alu.mult, accum_out=o(Q_SJ),
                )
                nc.vector.scalar_tensor_tensor(
                    out=scr2, in0=scr, scalar=1.0, in1=w1,
                    op0=Alu.mult, op1=Alu.mult, accum_out=o(Q_SJ2),
                )
                nc.vector.scalar_tensor_tensor(
                    out=scr, in0=cs, scalar=1.0, in1=w1,
                    op0=Alu.mult, op1=Alu.mult, accum_out=o(Q_SK),
                )
                nc.vector.scalar_tensor_tensor(
                    out=scr2, in0=scr, scalar=1.0, in1=w1,
                    op0=Alu.mult, op1=Alu.mult, accum_out=o(Q_SK2),
                )
                # jstar: first row whose rowmax equals the batch max.
                # ge = (rm >= M); rj = max(ge * (64-j)); jstar = 64 - rj
                ge = spool.tile([P, H], F32, tag="ge")
                nc.vector.tensor_scalar(
                    out=ge, in0=rm, scalar1=o(Q_M), scalar2=None, op0=Alu.is_ge
                )
                nc.vector.tensor_mul(out=scr, in0=ge, in1=wr)
                nc.vector.tensor_reduce(out=o(Q_RJ), in_=scr, axis=Ax.X, op=Alu.max)

            nc.sync.dma_start(out=out[:, :], in_=outs)

    _split_multiwait_instructions(nc)
    return nc


_cache: dict[int, bass.Bass] = {}


def _get(bpc: int) -> bass.Bass:
    if bpc not in _cache:
        _cache[bpc] = build(bpc)
    return _cache[bpc]


def _wconsts() -> np.ndarray:
    j = np.arange(W, dtype=np.float32)
    return np.stack([j, j * j, (W - j).astype(np.float32)])


def _prepare(tensor: np.ndarray):
    t = np.ascontiguousarray(np.asarray(tensor), dtype=np.float32)
    bt = t.shape[0]
    bpc = bt // NCORES
    nc = _get(bpc)
    wc = _wconsts()
    in_maps = [
        {"x": t[c * bpc : (c + 1) * bpc], "wconsts": wc} for c in range(NCORES)
    ]
    return nc, in_maps, t


def _postprocess(t: np.ndarray, results: list[dict]) -> np.ndarray:
    bt = t.shape[0]
    bpc = bt // NCORES
    ms = []
    for c in range(NCORES):
        m = results[c]["moments"].reshape(P, bpc // P, 8)
        ms.append(m.transpose(1, 0, 2).reshape(bpc, 8))
    m = np.concatenate(ms, 0).astype(np.float64)  # [B, 8]

    S = m[:, Q_S]
    Sj = m[:, Q_SJ]
    Sj2 = m[:, Q_SJ2]
    Sk = m[:, Q_SK]
    Sk2 = m[:, Q_SK2]
    jstar = np.rint(W - m[:, Q_RJ]).astype(np.int64)

    # resolve kstar with exact first-occurrence semantics on the argmax row
    rows = t[np.arange(bt), jstar, :]  # [B, W]
    mrow = rows.max(axis=1)
    kstar = (rows == mrow[:, None]).argmax(axis=1)

    js = jstar.astype(np.float64)
    ks = kstar.astype(np.float64)
    loss = ((js * js + ks * ks) * S - 2.0 * js * Sj - 2.0 * ks * Sk + Sj2 + Sk2).sum()
    return np.asarray([loss], dtype=np.float32)


def kernel(tensor: np.ndarray) -> np.ndarray:
    nc, in_maps, t = _prepare(tensor)
    res = run_bass_kernel_spmd(nc, in_maps, list(range(NCORES)))
    return _postprocess(t, res.results)


# revision 9
# speedup vs baseline: 354.8094x; 354.8094x over previous
"""Trainium2 Bass kernel for the argmax-distance-weighted loss.

loss = sum_b sum_{j,k} ((jstar_b - j)^2 + (kstar_b - k)^2) * t[b,j,k]
where (jstar_b, kstar_b) is the (first-occurrence) argmax location of t[b].

Decomposition used per batch b:
    loss_b = (jstar^2 + kstar^2)*S - 2*jstar*Sj - 2*kstar*Sk + Sj2 + Sk2
with S    = sum t[b]
     Sj   = sum_j j   * rowsum[b, j]      rowsum[b,j] = sum_k t[b,j,k]
     Sj2  = sum_j j^2 * rowsum[b, j]
     Sk   = sum_k k   * colsum[b, k]      colsum[b,k] = sum_j t[b,j,k]
     Sk2  = sum_k k^2 * colsum[b, k]

Device (8 NeuronCores, data-parallel over batch): per 128-batch tile the
DVE does three full reduction passes (rowsum, colsum contiguous/strided,
rowmax) plus tiny fused weighted reductions, emitting 8 moments per batch.
jstar is recovered exactly (first row whose rowmax equals the batch max).
Host: gathers row jstar per batch (64 floats) to resolve kstar with exact
first-occurrence semantics, then evaluates the closed form and sums.
"""

import os
import sys

import numpy as np

try:
    import concourse.bass as bass
except ModuleNotFoundError:  # make concourse importable in a bare container
    for _p in ("/opt/trn_rl_repo", "/root/.axon_site/_ro/trn_rl_repo"):
        if os.path.isdir(_p) and _p not in sys.path:
            sys.path.insert(0, _p)
    import concourse.bass as bass

import concourse.mybir as mybir
from concourse.bass_utils import run_bass_kernel_spmd
from concourse.tile import TileContext
# --- workaround: this walrus build encodes only ONE sync-wait per TPB ---
# instruction. Tile attaches several waits to one instruction (tail drain,
# DMA copies, ...), which codegen rejects with "Too many sync wait
# commands". Post-pass: hoist all but the last wait of each instruction
# into standalone same-engine NoOps placed immediately before it.


def _split_multiwait_instructions(nc: bass.Bass) -> None:
    # (bb, inst-name) pairs needing surgery
    targets = []
    for fn in nc.m.functions:
        for bb in fn.blocks:
            for inst in bb.instructions:
                si = inst.sync_info
                if si is not None and len(si.on_wait) > 1:
                    targets.append((bb, inst.name))
    if not targets:
        return

    moved_nop_names: set[str] = set()
    plan: dict[str, list] = {}  # target-inst-name -> nop instructions
    for bb, iname in targets:
        inst = next(i for i in bb.instructions if i.name == iname)
        waits = list(inst.sync_info.on_wait)
        inst.sync_info.on_wait = waits[-1:]
        nops = []
        for w in waits[:-1]:
            bi = nc.engines[inst.engine].nop(nofuse=True, hint="split_wait")
            bi.ins.sync_info = mybir.SyncInfo(on_wait=[w], on_update=[])
            nops.append(bi.ins)
            moved_nop_names.add(bi.ins.name)
        plan[iname] = nops

    # relocate the nops to sit immediately before their target instruction
    for fn in nc.m.functions:
        for bb in fn.blocks:
            insts = list(bb.instructions)
            kept = [i for i in insts if i.name not in moved_nop_names]
            out: list = []
            changed = len(kept) != len(insts)
            for inst in kept:
                if inst.name in plan:
                    out.extend(plan[inst.name])
                    changed = True
                out.append(inst)
            if changed:
                bb.instructions = out

B, H, W = 8192, 64, 64
NCORES = 8
P = 128  # SBUF partitions

F32 = mybir.dt.float32
Alu = mybir.AluOpType
Ax = mybir.AxisListType

# output layout per (partition, tile): 8 fp32 moments
Q_M, Q_S, Q_SJ, Q_SJ2, Q_SK, Q_SK2, Q_RJ, Q_SPARE = range(8)


def build(bpc: int, repeats: int = 1) -> bass.Bass:
    """Build the per-core Bass program for `bpc` batches per core.

    `repeats` re-runs the whole pipeline N times in one program — used only
    for timing (slope method cancels the host dispatch overhead)."""
    ntiles = bpc // P
    assert ntiles * P == bpc

    nc = bass.Bass()
    x = nc.declare_dram_parameter("x", [bpc, H, W], F32, isOutput=False)
    wc = nc.declare_dram_parameter("wconsts", [3, W], F32, isOutput=False)
    out = nc.declare_dram_parameter("moments", [P, ntiles * 8], F32, isOutput=True)

    with TileContext(nc) as tc:
        with (
            tc.tile_pool(name="xpool", bufs=3) as xpool,
            tc.tile_pool(name="consts", bufs=1) as cpool,
            tc.tile_pool(name="inter", bufs=1) as ipool,
            tc.tile_pool(name="small", bufs=4) as spool,
        ):
            # Broadcast the weight constants [3, W] across all partitions.
            wtile = cpool.tile([P, 3, W], F32)
            wc_ap = wc[:, :]
            bcast = bass.AP(
                tensor=wc_ap.tensor,
                offset=wc_ap.offset,
                ap=[[0, P]] + list(wc_ap.ap),
            )
            nc.sync.dma_start(out=wtile, in_=bcast)
            w1 = wtile[:, 0, :]  # 0..63
            wr = wtile[:, 2, :]  # 64-j

            outs = ipool.tile([P, ntiles * 8], F32)
            nc.vector.memset(outs, 0.0)

            for _rep, t in ((r, t) for r in range(repeats) for t in range(ntiles)):
                xt = xpool.tile([P, H, W], F32)
                nc.sync.dma_start(out=xt, in_=x[t * P : (t + 1) * P, :, :])

                rs = spool.tile([P, H], F32, tag="rs")
                cs = spool.tile([P, W], F32, tag="cs")
                rm = spool.tile([P, H], F32, tag="rm")
                scr = spool.tile([P, H], F32, tag="scr")
                scr2 = spool.tile([P, H], F32, tag="scr2")

                # full-data passes
                nc.vector.tensor_reduce(out=rs, in_=xt[:, :, :], axis=Ax.X, op=Alu.add)
                nc.vector.tensor_reduce(out=rm, in_=xt[:, :, :], axis=Ax.X, op=Alu.max)
                xk = xt[:, :, :].rearrange("p j k -> p k j")
                nc.vector.tensor_reduce(out=cs, in_=xk, axis=Ax.X, op=Alu.add)

                def o(q, t=t):
                    return outs[:, t * 8 + q : t * 8 + q + 1]

                # batch max / total sum
                nc.vector.tensor_reduce(out=o(Q_M), in_=rm, axis=Ax.X, op=Alu.max)
                nc.vector.tensor_reduce(out=o(Q_S), in_=rs, axis=Ax.X, op=Alu.add)
                # weighted moments: out = (in0 * 1.0) * w; accum_out = sum(out)
                nc.vector.scalar_tensor_tensor(
                    out=scr, in0=rs, scalar=1.0, in1=w1,
                    op0=Alu.mult, op1=Alu.mult, accum_out=o(Q_SJ),
                )
                nc.vector.scalar_tensor_tensor(
                    out=scr2, in0=scr, scalar=1.0, in1=w1,
                    op0=Alu.mult, op1=Alu.mult, accum_out=o(Q_SJ2),
                )
                nc.vector.scalar_tensor_tensor(
                    out=scr, in0=cs, scalar=1.0, in1=w1,
                    op0=Alu.mult, op1=Alu.mult, accum_out=o(Q_SK),
                )
                nc.vector.scalar_tensor_tensor(
                    out=scr2, in0=scr, scalar=1.0, in1=w1,
                    op0=Alu.mult, op1=Alu.mult, accum_out=o(Q_SK2),
                )
                # jstar: first row whose rowmax equals the batch max.
                # ge = (rm >= M); rj = max(ge * (64-j)); jstar = 64 - rj
                ge = spool.tile([P, H], F32, tag="ge")
                nc.vector.tensor_scalar(
                    out=ge, in0=rm, scalar1=o(Q_M), scalar2=None, op0=Alu.is_ge
                )
                nc.vector.tensor_mul(out=scr, in0=ge, in1=wr)
                nc.vector.tensor_reduce(out=o(Q_RJ), in_=scr, axis=Ax.X, op=Alu.max)

            nc.sync.dma_start(out=out[:, :], in_=outs)

    _split_multiwait_instructions(nc)
    return nc


_cache: dict[int, bass.Bass] = {}


def _get(bpc: int) -> bass.Bass:
    if bpc not in _cache:
        _cache[bpc] = build(bpc)
    return _cache[bpc]


def _wconsts() -> np.ndarray:
    j = np.arange(W, dtype=np.float32)
    return np.stack([j, j * j, (W - j).astype(np.float32)])


def _prepare(tensor: np.ndarray):
    t = np.ascontiguousarray(np.asarray(tensor), dtype=np.float32)
    bt = t.shape[0]
    bpc = bt // NCORES
    nc = _get(bpc)
    wc = _wconsts()
    in_maps = [
        {"x": t[c * bpc : (c + 1) * bpc], "wconsts": wc} for c in range(NCORES)
    ]
    return nc, in_maps, t


def _postprocess(t: np.ndarray, results: list[dict]) -> np.ndarray:
    bt = t.shape[0]
    bpc = bt // NCORES
    ms = []
    for c in range(NCORES):
        m = results[c]["moments"].reshape(P, bpc // P, 8)
        ms.append(m.transpose(1, 0, 2).reshape(bpc, 8))
    m = np.concatenate(ms, 0).astype(np.float64)  # [B, 8]

    S = m[:, Q_S]
    Sj = m[:, Q_SJ]
    Sj2 = m[:, Q_SJ2]
    Sk = m[:, Q_SK]
    Sk2 = m[:, Q_SK2]
    jstar = np.rint(W - m[:, Q_RJ]).astype(np.int64)

    # resolve kstar with exact first-occurrence semantics on the argmax row
    rows = t[np.arange(bt), jstar, :]  # [B, W]
    mrow = rows.max(axis=1)
    kstar = (rows == mrow[:, None]).argmax(axis=1)

    js = jstar.astype(np.float64)
    ks = kstar.astype(np.float64)
    loss = ((js * js + ks * ks) * S - 2.0 * js * Sj - 2.0 * ks * Sk + Sj2 + Sk2).sum()
    return np.asarray([loss], dtype=np.float32)


def kernel(tensor: np.ndarray) -> np.ndarray:
    nc, in_maps, t = _prepare(tensor)
    res = run_bass_kernel_spmd(nc, in_maps, list(range(NCORES)))
    return _postprocess(t, res.results)
